# revision 10
# baseline (speedup 1.0000x reference)
"""MoE (top-2, 8 experts, capacity-truncated) forward on 8 Trainium2 NeuronCores.

Strategy (expert-parallel):
  - Host (CPU jax, bit-exact with the reference): router matmul + softmax +
    top-k, capacity truncation, per-expert token gather, aux-loss; the final
    weighted scatter-add combine (incl. the b2 bias term) also runs on host.
  - Device (Bass/Tile, one expert per core): the heavy grouped MLP
        y_e = gelu(xg_e @ w1_e + b1_e) @ w2_e
    over C_pad token slots per expert (C_pad = max expert load padded up,
    typically ~1280 of the 2560 capacity -> half the reference FLOPs).
    Phase 1 runs in float32r (TF32-like 11-bit mantissa, full PE rate);
    phase 2 runs in float16 (10-bit mantissa, full PE rate, half the DMA).
    Weight streams are spread over both HWDGE queues (sync + scalar) with
    activations/outputs on the gpsimd SWDGE queue.

Self-contained: hardcodes all shapes; no file reads.
"""

import os
import numpy as np

N, D, H, E, TOP_K = 4096, 2048, 8192, 8, 2
CAPACITY_FACTOR = 1.25
CAPACITY = max(int(CAPACITY_FACTOR * N / TOP_K), 1)  # 2560
P = 128
KC = D // P   # 16 contraction chunks for phase 1
MH = H // P   # 64 hidden blocks
DC = D // P   # 16 output-dim blocks for phase 2
N_CORES = 8

# info about the last device run, for the test harness
LAST_RUN_INFO = {}

_NC_CACHE = {}


def _round_f32r(a: np.ndarray) -> np.ndarray:
    """Round fp32 to fp32r (1s/8e/11m) with round-to-nearest-even, so the
    PE's truncation to the top 20 bits sees canonical values."""
    u = np.ascontiguousarray(a, dtype=np.float32).view(np.uint32)
    lsb = (u >> np.uint32(12)) & np.uint32(1)
    r = (u + np.uint32(0x7FF) + lsb) & np.uint32(0xFFFFF000)
    return r.view(np.float32)


def _token_tiles(c_pad: int) -> list[int]:
    """Split C_pad (multiple of 128, >=512) into rounds of <=768 tokens whose
    psum chunks are all >=256 (keeps f32r matmuls at full rate)."""
    tiles = [768] * (c_pad // 768)
    rem = c_pad - 768 * len(tiles)
    if rem == 128:
        tiles.pop()          # 768 + 128 -> 512 + 384
        tiles += [512, 384]
    elif rem == 640:
        tiles += [384, 256]  # 640 would chunk into 512+128 (<256)
    elif rem:
        tiles.append(rem)    # 256 / 384 / 512
    return tiles


def _chunks(tc: int) -> list[tuple[int, int]]:
    out, lo = [], 0
    while lo < tc:
        sz = 512 if tc - lo >= 512 else tc - lo
        out.append((lo, sz))
        lo += sz
    return out


def _build(c_pad: int):
    import concourse.mybir as mybir
    import concourse.tile as tile
    from concourse import bacc

    f32 = mybir.dt.float32
    f32r = mybir.dt.float32r
    f16 = mybir.dt.float16
    GELU = mybir.ActivationFunctionType.Gelu

    tiles = _token_tiles(c_pad)
    TCMAX = max(tiles)
    nc = bacc.Bacc()
    with tile.TileContext(nc) as tc:
        with tc.tile_pool(name="dram", bufs=1, space="DRAM") as dram:
            xT_d = dram.tile([P, KC, c_pad], f32r, kind="ExternalInput",
                             name="xT", uniquify=False)
            w1_d = dram.tile([P, MH, KC, P], f32r, kind="ExternalInput",
                             name="w1", uniquify=False)
            b1_d = dram.tile([P, MH], f32, kind="ExternalInput",
                             name="b1", uniquify=False)
            w2_d = dram.tile([P, DC, MH, P], f16, kind="ExternalInput",
                             name="w2", uniquify=False)
            # y stored transposed: y_d[p, dc, c] = y[c, dc*128+p]
            y_d = dram.tile([P, DC, c_pad], f32, kind="ExternalOutput",
                            name="y", uniquify=False)

            with (
                tc.tile_pool(name="cst", bufs=1) as cst,
                tc.tile_pool(name="xT", bufs=1) as xT_pool,
                tc.tile_pool(name="hT", bufs=1) as hT_pool,
                tc.tile_pool(name="w1s", bufs=3) as w1_pool,
                tc.tile_pool(name="w2s", bufs=3) as w2_pool,
                tc.tile_pool(name="yev", bufs=2) as y_pool,
                tc.tile_pool(name="ps1", bufs=2, space="PSUM") as ps1,
                tc.tile_pool(name="ps2", bufs=2, space="PSUM") as ps2,
            ):
                b1 = cst.tile([P, MH], f32, name="b1_sb")
                nc.gpsimd.dma_start(out=b1[:], in_=b1_d[:])

                c0 = 0
                for t, TC in enumerate(tiles):
                    chunks = _chunks(TC)
                    # per-round token slice [c0, c0+TC)
                    xT = xT_pool.tile([P, KC, TCMAX], f32r, name="xT_sb",
                                      tag="xT")[:, :, :TC]
                    if t == 0:
                        # round 0 gates the first matmuls: split by k across
                        # two queues so k=0 lands in ~2us instead of ~30us
                        for k in range(KC):
                            eng = nc.gpsimd if k % 2 == 0 else nc.scalar
                            eng.dma_start(out=xT[:, k, :],
                                          in_=xT_d[:, k, c0:c0 + TC])
                    else:
                        nc.gpsimd.dma_start(out=xT[:], in_=xT_d[:, :, c0:c0 + TC])
                    hT = hT_pool.tile([P, MH, TCMAX], f16, name="hT_sb",
                                      tag="hT")[:, :, :TC]

                    # ---- phase 1: hT[:, m, :] = gelu(w1[m].T @ x + b1[m]) ----
                    for m in range(MH):
                        w1t = w1_pool.tile([P, KC, P], f32r, name="w1t", tag="w1t")
                        nc.sync.dma_start(out=w1t[:], in_=w1_d[:, m, :, :])
                        pt = ps1.tile([P, TC], f32, name="pt1", tag="pt1",
                                      padded_shape=[P, 1024])
                        for (lo, sz) in chunks:
                            for k in range(KC):
                                nc.tensor.matmul(pt[:, lo:lo + sz],
                                                 w1t[:, k, :],
                                                 xT[:, k, lo:lo + sz],
                                                 start=(k == 0),
                                                 stop=(k == KC - 1))
                        nc.scalar.activation(hT[:, m, :], pt[:], GELU,
                                             bias=b1[:, m:m + 1])

                    # ---- phase 2 (transposed): yT[dc] = sum_m w2[m,dc].T @ hT[m] ----
                    MQ = 16  # hidden blocks per streamed w2 sub-tile
                    for dc in range(DC):
                        pt2 = ps2.tile([P, TC], f32, name="pt2", tag="pt2",
                                       padded_shape=[P, 1024])
                        for ms in range(MH // MQ):
                            w2t = w2_pool.tile([P, MQ, P], f16, name="w2t",
                                               tag="w2t")
                            nc.scalar.dma_start(
                                out=w2t[:],
                                in_=w2_d[:, dc, ms * MQ:(ms + 1) * MQ, :])
                            for mm in range(MQ):
                                m = ms * MQ + mm
                                for (lo, sz) in chunks:
                                    nc.tensor.matmul(pt2[:, lo:lo + sz],
                                                     w2t[:, mm, :],
                                                     hT[:, m, lo:lo + sz],
                                                     start=(m == 0),
                                                     stop=(m == MH - 1))
                        yt = y_pool.tile([P, TCMAX], f32, name="yt",
                                         tag="yt")[:, :TC]
                        nc.vector.tensor_copy(yt[:], pt2[:])
                        nc.gpsimd.dma_start(out=y_d[:, dc, c0:c0 + TC], in_=yt[:])
                    c0 += TC
    nc.compile()
    return nc


def _get_nc(c_pad: int):
    if c_pad not in _NC_CACHE:
        _NC_CACHE[c_pad] = _build(c_pad)
    return _NC_CACHE[c_pad]


def kernel(x, router_w, router_b, w1, b1, w2, b2):
    import jax
    import jax.numpy as jnp
    from concourse.bass_utils import run_bass_kernel_spmd

    x = np.ascontiguousarray(np.asarray(x, dtype=np.float32))
    router_w = np.asarray(router_w, dtype=np.float32)
    router_b = np.asarray(router_b, dtype=np.float32)
    w1 = np.asarray(w1, dtype=np.float32)
    b1 = np.asarray(b1, dtype=np.float32)
    w2 = np.asarray(w2, dtype=np.float32)
    b2 = np.asarray(b2, dtype=np.float32)

    # ---- router + dispatch plan on host CPU, bit-exact with the reference ----
    cpu = jax.devices("cpu")[0]
    with jax.default_device(cpu):
        xj = jnp.asarray(x)
        logits = xj @ jnp.asarray(router_w) + jnp.asarray(router_b)
        probs = jax.nn.softmax(logits, axis=-1)
        ew, ei = jax.lax.top_k(probs, TOP_K)                   # [N, K]
        eids = jnp.arange(E)
        hit = ei[None, :, :] == eids[:, None, None]            # [E, N, K]
        sel = hit.any(-1)                                      # [E, N]
        w_tok = jnp.sum(jnp.where(hit, ew[None], 0.0), axis=-1)  # [E, N]
        rank = jnp.cumsum(sel, axis=1) - 1
        keep = sel & (rank < CAPACITY)
        pos = jnp.where(keep, jnp.arange(N)[None, :], N)
        idx = jnp.sort(pos, axis=1)[:, :CAPACITY]              # [E, C]
        valid = idx < N
        idx_c = jnp.where(valid, idx, 0)
        # aux losses (world_size = 1)
        expert_count = sel.sum(axis=1).astype(probs.dtype)
        usage = expert_count / N
        density = probs.mean(axis=0)
        balance_loss = jnp.sum(density * usage) * E
        important = probs.sum(axis=0)
        important_loss = jnp.mean(important ** 2)
        aux_loss = balance_loss + important_loss

        idx_c = np.asarray(idx_c)
        valid = np.asarray(valid)
        w_tok = np.asarray(w_tok)
        aux_loss = np.asarray(aux_loss)

    counts = valid.sum(axis=1).astype(np.int64)                # [E]
    c_max = int(counts.max()) if counts.max() > 0 else 1
    c_pad = min(-(-c_max // 128) * 128, CAPACITY)
    c_pad = max(c_pad, 512)

    # ---- per-core (per-expert) device inputs ----
    in_maps = []
    for e in range(E):
        cnt = int(counts[e])
        idx_e = idx_c[e, :cnt]
        xp = np.zeros((c_pad, D), dtype=np.float32)
        xp[:cnt] = x[idx_e]
        xT = np.ascontiguousarray(
            _round_f32r(xp).T.reshape(KC, P, c_pad).transpose(1, 0, 2))
        # w1[e]: [D, H] -> [P, MH, KC, P] with w1_dev[p, m, k, q] = w1[k*128+p, m*128+q]
        w1_dev = np.ascontiguousarray(
            _round_f32r(w1[e]).reshape(KC, P, MH, P).transpose(1, 2, 0, 3))
        # w2[e]: [H, D] -> [P, DC, MH, P] fp16 with w2_dev[p, dc, m, q] = w2[m*128+p, dc*128+q]
        w2_dev = np.ascontiguousarray(
            w2[e].reshape(MH, P, DC, P).transpose(1, 2, 0, 3).astype(np.float16))
        b1_dev = np.ascontiguousarray(b1[e].reshape(MH, P).T)
        in_maps.append({"xT": xT, "w1": w1_dev, "b1": b1_dev, "w2": w2_dev})

    nc = _get_nc(c_pad)
    trace = bool(int(os.environ.get("KERNEL_TRACE", "0")))
    res = run_bass_kernel_spmd(nc, in_maps, core_ids=list(range(N_CORES)),
                               trace=trace)
    LAST_RUN_INFO.clear()
    LAST_RUN_INFO.update({
        "exec_time_ns": res.exec_time_ns,
        "mean_exec_time_ns": res.mean_exec_time_ns,
        "c_pad": c_pad,
        "counts": counts,
        "instructions_and_trace": res.instructions_and_trace,
        "per_core_scope_times": res.per_core_scope_times,
    })

    # ---- combine on host: final[t] += wg * (y_slot + b2[e]) ----
    final = np.zeros((N, D), dtype=np.float32)
    for e in range(E):
        cnt = int(counts[e])
        if cnt == 0:
            continue
        y_dev = res.results[e]["y"]                  # [P, DC, c_pad], transposed
        y = y_dev.transpose(2, 1, 0).reshape(c_pad, D)[:cnt]
        idx_e = idx_c[e, :cnt]
        wg = w_tok[e, idx_e].astype(np.float32)[:, None]
        final[idx_e] += wg * (y + b2[e][None, :])
    return final, np.float32(aux_loss)


# revision 11
# speedup vs baseline: 1.0700x; 1.0700x over previous
"""MoE (top-2, 8 experts, capacity-truncated) forward on 8 Trainium2 NeuronCores.

Strategy (expert-parallel):
  - Host (CPU jax, bit-exact with the reference): router matmul + softmax +
    top-k, capacity truncation, per-expert token gather, aux-loss; the final
    weighted scatter-add combine (incl. the b2 bias term) also runs on host.
  - Device (Bass/Tile, one expert per core): the heavy grouped MLP
        y_e = gelu(xg_e @ w1_e + b1_e) @ w2_e
    over C_pad token slots per expert (C_pad = max expert load padded up,
    typically ~1280 of the 2560 capacity -> half the reference FLOPs).
    Phase 1 runs in float32r (TF32-like 11-bit mantissa, full PE rate);
    phase 2 runs in float16 (10-bit mantissa, full PE rate, half the DMA).
    Weight streams are spread over both HWDGE queues (sync + scalar) with
    activations/outputs on the gpsimd SWDGE queue.

Self-contained: hardcodes all shapes; no file reads.
"""

import os
import numpy as np

N, D, H, E, TOP_K = 4096, 2048, 8192, 8, 2
CAPACITY_FACTOR = 1.25
CAPACITY = max(int(CAPACITY_FACTOR * N / TOP_K), 1)  # 2560
P = 128
KC = D // P   # 16 contraction chunks for phase 1
MH = H // P   # 64 hidden blocks
DC = D // P   # 16 output-dim blocks for phase 2
N_CORES = 8

# info about the last device run, for the test harness
LAST_RUN_INFO = {}

_NC_CACHE = {}


def _round_f32r(a: np.ndarray) -> np.ndarray:
    """Round fp32 to fp32r (1s/8e/11m) with round-to-nearest-even, so the
    PE's truncation to the top 20 bits sees canonical values."""
    u = np.ascontiguousarray(a, dtype=np.float32).view(np.uint32)
    lsb = (u >> np.uint32(12)) & np.uint32(1)
    r = (u + np.uint32(0x7FF) + lsb) & np.uint32(0xFFFFF000)
    return r.view(np.float32)


def _token_tiles(c_pad: int) -> list[int]:
    """Split C_pad (multiple of 128, >=512) into rounds of <=768 tokens whose
    psum chunks are all >=256 (keeps f32r matmuls at full rate)."""
    tiles = [768] * (c_pad // 768)
    rem = c_pad - 768 * len(tiles)
    if rem == 128:
        tiles.pop()          # 768 + 128 -> 512 + 384
        tiles += [512, 384]
    elif rem == 640:
        tiles += [384, 256]  # 640 would chunk into 512+128 (<256)
    elif rem:
        tiles.append(rem)    # 256 / 384 / 512
    return tiles


def _chunks(tc: int) -> list[tuple[int, int]]:
    out, lo = [], 0
    while lo < tc:
        sz = 512 if tc - lo >= 512 else tc - lo
        out.append((lo, sz))
        lo += sz
    return out


def _build(c_pad: int):
    import concourse.mybir as mybir
    import concourse.tile as tile
    from concourse import bacc

    f32 = mybir.dt.float32
    f32r = mybir.dt.float32r
    f16 = mybir.dt.float16
    GELU = mybir.ActivationFunctionType.Gelu

    tiles = _token_tiles(c_pad)
    TCMAX = max(tiles)
    nc = bacc.Bacc()
    with tile.TileContext(nc) as tc:
        with tc.tile_pool(name="dram", bufs=1, space="DRAM") as dram:
            xT_d = dram.tile([P, KC, c_pad], f32r, kind="ExternalInput",
                             name="xT", uniquify=False)
            w1_d = dram.tile([P, MH, KC, P], f32r, kind="ExternalInput",
                             name="w1", uniquify=False)
            b1_d = dram.tile([P, MH], f32, kind="ExternalInput",
                             name="b1", uniquify=False)
            w2_d = dram.tile([P, DC, MH, P], f16, kind="ExternalInput",
                             name="w2", uniquify=False)
            # y stored transposed: y_d[p, dc, c] = y[c, dc*128+p]
            y_d = dram.tile([P, DC, c_pad], f32, kind="ExternalOutput",
                            name="y", uniquify=False)

            with (
                tc.tile_pool(name="cst", bufs=1) as cst,
                tc.tile_pool(name="xT", bufs=1) as xT_pool,
                tc.tile_pool(name="hT", bufs=1) as hT_pool,
                tc.tile_pool(name="w1s", bufs=4) as w1_pool,
                tc.tile_pool(name="w2s", bufs=4) as w2_pool,
                tc.tile_pool(name="yev", bufs=2) as y_pool,
                tc.tile_pool(name="ps1", bufs=2, space="PSUM") as ps1,
                tc.tile_pool(name="ps2", bufs=2, space="PSUM") as ps2,
            ):
                b1 = cst.tile([P, MH], f32, name="b1_sb")

                c0 = 0
                for t, TC in enumerate(tiles):
                    chunks = _chunks(TC)
                    # per-round token slice [c0, c0+TC)
                    xT = xT_pool.tile([P, KC, TCMAX], f32r, name="xT_sb",
                                      tag="xT")[:, :, :TC]
                    if t == 0:
                        # round 0 gates the first matmuls: split by k across
                        # two queues so k=0 lands in ~2us instead of ~30us
                        for k in range(KC):
                            eng = nc.gpsimd if k % 2 == 0 else nc.scalar
                            eng.dma_start(out=xT[:, k, :],
                                          in_=xT_d[:, k, c0:c0 + TC])
                        nc.gpsimd.dma_start(out=b1[:], in_=b1_d[:])
                    else:
                        nc.gpsimd.dma_start(out=xT[:], in_=xT_d[:, :, c0:c0 + TC])
                    hT = hT_pool.tile([P, MH, TCMAX], f16, name="hT_sb",
                                      tag="hT")[:, :, :TC]

                    # ---- phase 1: hT[:, m, :] = gelu(w1[m].T @ x + b1[m]) ----
                    for m in range(MH):
                        w1t = w1_pool.tile([P, KC, P], f32r, name="w1t", tag="w1t")
                        nc.sync.dma_start(out=w1t[:], in_=w1_d[:, m, :, :])
                        pt = ps1.tile([P, TC], f32, name="pt1", tag="pt1",
                                      padded_shape=[P, 1024])
                        for (lo, sz) in chunks:
                            for k in range(KC):
                                nc.tensor.matmul(pt[:, lo:lo + sz],
                                                 w1t[:, k, :],
                                                 xT[:, k, lo:lo + sz],
                                                 start=(k == 0),
                                                 stop=(k == KC - 1))
                        nc.scalar.activation(hT[:, m, :], pt[:], GELU,
                                             bias=b1[:, m:m + 1])

                    # ---- phase 2 (transposed): yT[dc] = sum_m w2[m,dc].T @ hT[m] ----
                    MQ = 16  # hidden blocks per streamed w2 sub-tile
                    for dc in range(DC):
                        pt2 = ps2.tile([P, TC], f32, name="pt2", tag="pt2",
                                       padded_shape=[P, 1024])
                        for ms in range(MH // MQ):
                            w2t = w2_pool.tile([P, MQ, P], f16, name="w2t",
                                               tag="w2t")
                            nc.scalar.dma_start(
                                out=w2t[:],
                                in_=w2_d[:, dc, ms * MQ:(ms + 1) * MQ, :])
                            for mm in range(MQ):
                                m = ms * MQ + mm
                                for (lo, sz) in chunks:
                                    nc.tensor.matmul(pt2[:, lo:lo + sz],
                                                     w2t[:, mm, :],
                                                     hT[:, m, lo:lo + sz],
                                                     start=(m == 0),
                                                     stop=(m == MH - 1))
                        yt = y_pool.tile([P, TCMAX], f32, name="yt",
                                         tag="yt")[:, :TC]
                        nc.vector.tensor_copy(yt[:], pt2[:])
                        nc.gpsimd.dma_start(out=y_d[:, dc, c0:c0 + TC], in_=yt[:])
                    c0 += TC
    nc.compile()
    return nc


def _get_nc(c_pad: int):
    if c_pad not in _NC_CACHE:
        _NC_CACHE[c_pad] = _build(c_pad)
    return _NC_CACHE[c_pad]


def kernel(x, router_w, router_b, w1, b1, w2, b2):
    import jax
    import jax.numpy as jnp
    from concourse.bass_utils import run_bass_kernel_spmd

    x = np.ascontiguousarray(np.asarray(x, dtype=np.float32))
    router_w = np.asarray(router_w, dtype=np.float32)
    router_b = np.asarray(router_b, dtype=np.float32)
    w1 = np.asarray(w1, dtype=np.float32)
    b1 = np.asarray(b1, dtype=np.float32)
    w2 = np.asarray(w2, dtype=np.float32)
    b2 = np.asarray(b2, dtype=np.float32)

    # ---- router + dispatch plan on host CPU, bit-exact with the reference ----
    cpu = jax.devices("cpu")[0]
    with jax.default_device(cpu):
        xj = jnp.asarray(x)
        logits = xj @ jnp.asarray(router_w) + jnp.asarray(router_b)
        probs = jax.nn.softmax(logits, axis=-1)
        ew, ei = jax.lax.top_k(probs, TOP_K)                   # [N, K]
        eids = jnp.arange(E)
        hit = ei[None, :, :] == eids[:, None, None]            # [E, N, K]
        sel = hit.any(-1)                                      # [E, N]
        w_tok = jnp.sum(jnp.where(hit, ew[None], 0.0), axis=-1)  # [E, N]
        rank = jnp.cumsum(sel, axis=1) - 1
        keep = sel & (rank < CAPACITY)
        pos = jnp.where(keep, jnp.arange(N)[None, :], N)
        idx = jnp.sort(pos, axis=1)[:, :CAPACITY]              # [E, C]
        valid = idx < N
        idx_c = jnp.where(valid, idx, 0)
        # aux losses (world_size = 1)
        expert_count = sel.sum(axis=1).astype(probs.dtype)
        usage = expert_count / N
        density = probs.mean(axis=0)
        balance_loss = jnp.sum(density * usage) * E
        important = probs.sum(axis=0)
        important_loss = jnp.mean(important ** 2)
        aux_loss = balance_loss + important_loss

        idx_c = np.asarray(idx_c)
        valid = np.asarray(valid)
        w_tok = np.asarray(w_tok)
        aux_loss = np.asarray(aux_loss)

    counts = valid.sum(axis=1).astype(np.int64)                # [E]
    c_max = int(counts.max()) if counts.max() > 0 else 1
    c_pad = min(-(-c_max // 128) * 128, CAPACITY)
    c_pad = max(c_pad, 512)

    # ---- per-core (per-expert) device inputs ----
    in_maps = []
    for e in range(E):
        cnt = int(counts[e])
        idx_e = idx_c[e, :cnt]
        xp = np.zeros((c_pad, D), dtype=np.float32)
        xp[:cnt] = x[idx_e]
        xT = np.ascontiguousarray(
            _round_f32r(xp).T.reshape(KC, P, c_pad).transpose(1, 0, 2))
        # w1[e]: [D, H] -> [P, MH, KC, P] with w1_dev[p, m, k, q] = w1[k*128+p, m*128+q]
        w1_dev = np.ascontiguousarray(
            _round_f32r(w1[e]).reshape(KC, P, MH, P).transpose(1, 2, 0, 3))
        # w2[e]: [H, D] -> [P, DC, MH, P] fp16 with w2_dev[p, dc, m, q] = w2[m*128+p, dc*128+q]
        w2_dev = np.ascontiguousarray(
            w2[e].reshape(MH, P, DC, P).transpose(1, 2, 0, 3).astype(np.float16))
        b1_dev = np.ascontiguousarray(b1[e].reshape(MH, P).T)
        in_maps.append({"xT": xT, "w1": w1_dev, "b1": b1_dev, "w2": w2_dev})

    nc = _get_nc(c_pad)
    trace = bool(int(os.environ.get("KERNEL_TRACE", "0")))
    res = run_bass_kernel_spmd(nc, in_maps, core_ids=list(range(N_CORES)),
                               trace=trace)
    LAST_RUN_INFO.clear()
    LAST_RUN_INFO.update({
        "exec_time_ns": res.exec_time_ns,
        "mean_exec_time_ns": res.mean_exec_time_ns,
        "c_pad": c_pad,
        "counts": counts,
        "instructions_and_trace": res.instructions_and_trace,
        "per_core_scope_times": res.per_core_scope_times,
    })

    # ---- combine on host: final[t] += wg * (y_slot + b2[e]) ----
    final = np.zeros((N, D), dtype=np.float32)
    for e in range(E):
        cnt = int(counts[e])
        if cnt == 0:
            continue
        y_dev = res.results[e]["y"]                  # [P, DC, c_pad], transposed
        y = y_dev.transpose(2, 1, 0).reshape(c_pad, D)[:cnt]
        idx_e = idx_c[e, :cnt]
        wg = w_tok[e, idx_e].astype(np.float32)[:, None]
        final[idx_e] += wg * (y + b2[e][None, :])
    return final, np.float32(aux_loss)
